# revision 23
# baseline (speedup 1.0000x reference)
"""DeepFM (embedding_lookup) Trainium2 Bass kernel.

Sharding: data-parallel on batch across 8 NeuronCores; the embedding
table is replicated per core.

Gather strategy: the generic SWDGE indirect DMA (InstDMACopy) consumes
only ONE index per partition per instruction (~1.3 us of serialized
Pool-engine time per 128 rows -> ~540 us for 53k rows). Instead we use
the ANT dma_gather ucode (InstDMAGatherAnt), which amortizes the fixed
cost over thousands of indices per instruction. Its constraints:
int16 indices and a 256B-multiple HBM row stride. So the table is
packed as bf16 quads: 4 vocab rows x 32 bf16 (17 real values:
16 emb + 1 emb_first, padded to 64B) = one 256B element, idx>>2 <
25000 fits int16. One gather per feature (26 per core, 2048 indices
each) followed by a mask-predicated 4-way selection of the right
quarter (masks are host-computed from idx&3).

All matmuls run in bf16 (1 cycle/row vs 4 for fp32).

Self-contained: hardcodes all shapes from the problem spec.
"""

import numpy as np
import ml_dtypes

import concourse.bass as bass
import concourse.bacc as bacc
import concourse.mybir as mybir
import concourse.tile as tile
from concourse.bass_utils import run_bass_kernel_spmd
from concourse.masks import make_identity

F32 = mybir.dt.float32
BF16 = mybir.dt.bfloat16
I16 = mybir.dt.int16
U8 = mybir.dt.uint8
AF = mybir.ActivationFunctionType
ALU = mybir.AluOpType
BFNP = ml_dtypes.bfloat16

# Problem dims
B, NCONT, F, V, D = 16384, 13, 26, 100000, 16
H1, H2 = 400, 400
NCORES = 8
BC = B // NCORES          # 2048 rows per core
SUB = 128                 # batch subtile (partition dim)
NSUB = 4                  # subtiles per block
BLK = SUB * NSUB          # 512 rows per block
NBLK = BC // BLK          # 4 blocks per core
NG = BC // SUB            # 16 groups (subtiles) per core
W17 = D + 1               # packed row: 16 emb + 1 emb_first
GW = F * W17              # 442 selected values per batch row
XW = GW + NCONT           # 455 = X' row width (f-major 17-wide | cont)
QV = V // 4               # 25000 quads per feature
QE = 128                  # bf16 elements per quad (4 rows x 32)


def _chunks(total, step=128):
    return [(s, min(step, total - s)) for s in range(0, total, step)]


# X' K-chunks for L1: emb chunks + a separate 13-row cont chunk (engine
# partition writes must start at 0/32/64/96, so cont can't share a chunk)
KCH = _chunks(GW) + [(GW, NCONT)]
MCH1 = _chunks(H1)         # L1 M-tiles == L2 K-chunks
MCH2 = _chunks(H2)         # L2 M-tiles == out-layer K-chunks


def build_kernel():
    import os
    kstage = os.environ.get("KSTAGE", "full")
    nc = bacc.Bacc("TRN2", target_bir_lowering=False, debug=False)

    t_ptab = nc.dram_tensor("ptab", [F, QV, QE], BF16, kind="ExternalInput")
    t_idx = nc.dram_tensor("idx16", [SUB, F * SUB], I16, kind="ExternalInput")
    t_msk = nc.dram_tensor("msk", [SUB, 4 * F * NG], U8, kind="ExternalInput")
    t_cont = nc.dram_tensor("cont3", [SUB, NG * NCONT], BF16, kind="ExternalInput")
    t_w1 = nc.dram_tensor("w1p", [XW, H1], BF16, kind="ExternalInput")
    t_w2 = nc.dram_tensor("w2", [H1, H2], BF16, kind="ExternalInput")
    t_b1 = nc.dram_tensor("b1", [H1, 1], F32, kind="ExternalInput")
    t_b2 = nc.dram_tensor("b2", [H2, 1], F32, kind="ExternalInput")
    t_wo = nc.dram_tensor("wo", [128, len(MCH2)], BF16, kind="ExternalInput")
    t_wc = nc.dram_tensor("wc", [128, NCONT], BF16, kind="ExternalInput")
    t_fs = nc.dram_tensor("fs", [1, 1], F32, kind="ExternalInput")
    t_idb = nc.dram_tensor("idb", [128, 128], BF16, kind="ExternalInput")
    t_idf = nc.dram_tensor("idf", [128, 128], F32, kind="ExternalInput")
    t_ob = nc.dram_tensor("ob", [1, 1], F32, kind="ExternalInput")
    t_y = nc.dram_tensor("y", [NBLK, 1, BLK], F32, kind="ExternalOutput")

    with tile.TileContext(nc) as tc:
        with (
            tc.tile_pool(name="wpool", bufs=1) as wpool,
            tc.tile_pool(name="gpool", bufs=3) as gpool,
            tc.tile_pool(name="xpool", bufs=2) as xpool,
            tc.tile_pool(name="hpool", bufs=2) as hpool,
            tc.tile_pool(name="fpool", bufs=2) as fpool,
            tc.tile_pool(name="opool", bufs=2) as opool,
            tc.tile_pool(name="pt_ps", bufs=2, space="PSUM") as pt_ps,
            tc.tile_pool(name="mm_ps", bufs=2, space="PSUM") as mm_ps,
            tc.tile_pool(name="o_ps", bufs=1, space="PSUM") as o_ps,
        ):
            # ---- indices first: gathers are the critical path ----
            idx_sb = wpool.tile([SUB, F * SUB], I16)
            nc.sync.dma_start(out=idx_sb[:], in_=t_idx[:])
            msk_sb = wpool.tile([SUB, 4 * F * NG], U8)
            nc.sync.dma_start(out=msk_sb[:], in_=t_msk[:])
            cont_sb = wpool.tile([SUB, NG * NCONT], BF16)
            nc.sync.dma_start(out=cont_sb[:], in_=t_cont[:])
            cont3 = cont_sb[:].rearrange("p (g c) -> p g c", c=NCONT)

            identb = wpool.tile([128, 128], BF16)
            nc.sync.dma_start(out=identb[:], in_=t_idb[:])
            identf = wpool.tile([128, 128], F32)
            nc.sync.dma_start(out=identf[:], in_=t_idf[:])

            w1_sb = []
            for ci, (k0, ks) in enumerate(KCH):
                w1c = wpool.tile([128, H1], BF16, name=f"w1c{ci}")
                nc.sync.dma_start(out=w1c[0:ks, :], in_=t_w1[k0 : k0 + ks, :])
                w1_sb.append(w1c)
            w2_sb = []
            for ci, (k0, ks) in enumerate(MCH1):
                w2c = wpool.tile([128, H2], BF16, name=f"w2c{ci}")
                nc.sync.dma_start(out=w2c[0:ks, :], in_=t_w2[k0 : k0 + ks, :])
                w2_sb.append(w2c)
            b1_sb = []
            for mi, (m0, ms) in enumerate(MCH1):
                b1m = wpool.tile([128, 1], F32, name=f"b1m{mi}")
                nc.sync.dma_start(out=b1m[0:ms, :], in_=t_b1[m0 : m0 + ms, :])
                b1_sb.append(b1m)
            b2_sb = []
            for mi, (m0, ms) in enumerate(MCH2):
                b2m = wpool.tile([128, 1], F32, name=f"b2m{mi}")
                nc.sync.dma_start(out=b2m[0:ms, :], in_=t_b2[m0 : m0 + ms, :])
                b2_sb.append(b2m)
            wo_sb = wpool.tile([128, len(MCH2)], BF16)
            nc.sync.dma_start(out=wo_sb[:], in_=t_wo[:])
            wc_sb = wpool.tile([128, NCONT], BF16)
            nc.sync.dma_start(out=wc_sb[:], in_=t_wc[:])
            fs_sb = wpool.tile([1, 1], F32)
            nc.sync.dma_start(out=fs_sb[:], in_=t_fs[:])
            ob_sb = wpool.tile([1, 1], F32)
            nc.sync.dma_start(out=ob_sb[:], in_=t_ob[:])

            # selected embeddings, batch-major: [p, group, f-major 17-wide]
            xall = wpool.tile([SUB, NG * GW], BF16)
            xall3 = xall[:].rearrange("p (g w) -> p g w", w=GW)

            # FM running sums, accumulated per feature during the gather
            # sweep so no FM work remains in the post-sweep tail
            seacc = wpool.tile([SUB, NG * D], F32)
            nc.vector.memset(seacc[:], 0.0)
            seacc3 = seacc[:].rearrange("p (g d) -> p g d", d=D)
            r2acc = wpool.tile([SUB, NG], F32)
            nc.vector.memset(r2acc[:], 0.0)
            rfacc = wpool.tile([SUB, NG], F32)
            nc.vector.memset(rfacc[:], 0.0)

            # ---- Phase A: per-feature gather + quad selection ----
            for f in range(F):
                g = gpool.tile([SUB, NG * QE], BF16, tag="g")
                g3 = g[:].rearrange("p (n e) -> p n e", e=QE)
                nc.gpsimd.dma_gather(
                    out_ap=g3,
                    in_ap=t_ptab[f],
                    idxs_ap=idx_sb[:, f * SUB : (f + 1) * SUB],
                    num_idxs=BC,
                    num_idxs_reg=BC,
                    elem_size=QE,
                    # single-packet concatenation overflows one DMA ring at
                    # 2048 descriptors and wedges the device
                    single_packet=False,
                )
                xf = xall3[:, :, f * W17 : (f + 1) * W17]
                for q in (0, 1, 2, 3):
                    mk = msk_sb[
                        :, q * F * NG + f * NG : q * F * NG + (f + 1) * NG
                    ]
                    mk3 = mk.unsqueeze(2).broadcast_to([SUB, NG, W17])
                    nc.vector.copy_predicated(
                        out=xf, mask=mk3, data=g3[:, :, 32 * q : 32 * q + W17]
                    )
                ef = xall3[:, :, f * W17 : f * W17 + D]
                nc.vector.tensor_add(out=seacc3, in0=seacc3, in1=ef)
                sqf = fpool.tile([SUB, NG * D], BF16, tag="sqf")
                sqf3 = sqf[:].rearrange("p (g d) -> p g d", d=D)
                nc.vector.tensor_mul(out=sqf3, in0=ef, in1=ef)
                sqr = fpool.tile([SUB, NG], F32, tag="sqr")
                nc.vector.tensor_reduce(
                    out=sqr[:], in_=sqf3, axis=mybir.AxisListType.X, op=ALU.add
                )
                nc.vector.tensor_add(out=r2acc[:], in0=r2acc[:], in1=sqr[:])
                firstc = xall3[:, :, f * W17 + D : f * W17 + D + 1].rearrange(
                    "p g w -> p (g w)"
                )
                nc.vector.tensor_add(out=rfacc[:], in0=rfacc[:], in1=firstc)

            # ---- FM epilogue: 0.5*(|se|^2 - r2) + cont.wc + rf ----
            se2 = wpool.tile([SUB, NG * D], F32)
            nc.vector.tensor_mul(out=se2[:], in0=seacc[:], in1=seacc[:])
            r1 = wpool.tile([SUB, NG], F32)
            nc.vector.tensor_reduce(
                out=r1[:], in_=se2[:].rearrange("p (g d) -> p g d", d=D),
                axis=mybir.AxisListType.X, op=ALU.add,
            )
            cw = wpool.tile([SUB, NG * NCONT], BF16)
            wc3 = wc_sb[:].unsqueeze(1).broadcast_to([SUB, NG, NCONT])
            nc.vector.tensor_mul(
                out=cw[:].rearrange("p (g c) -> p g c", c=NCONT),
                in0=cont3, in1=wc3)
            r3 = wpool.tile([SUB, NG], F32)
            nc.vector.tensor_reduce(
                out=r3[:], in_=cw[:].rearrange("p (g c) -> p g c", c=NCONT),
                axis=mybir.AxisListType.X, op=ALU.add,
            )
            t1 = wpool.tile([SUB, NG], F32)
            nc.vector.tensor_sub(out=t1[:], in0=r1[:], in1=r2acc[:])
            t2 = wpool.tile([SUB, NG], F32)
            nc.vector.tensor_scalar_mul(out=t2[:], in0=t1[:], scalar1=0.5)
            t3 = wpool.tile([SUB, NG], F32)
            nc.vector.tensor_add(out=t3[:], in0=t2[:], in1=r3[:])
            fmall = wpool.tile([SUB, NG], F32)
            nc.vector.tensor_add(out=fmall[:], in0=t3[:], in1=rfacc[:])

            # ---- Phase B: per-block transposes, FM, DNN ----
            for blk in range(NBLK):
                # transpose X' -> xT chunks [128, 512] (bf16).
                # X' rows: [emb f-major 0:442 | cont 442:455]; the last
                # chunk merges a 58-row emb transpose and a 13-row cont
                # transpose (separate PSUM tiles: matmul outputs must
                # start at partition 0).
                xt_sb = []
                for ci, (k0, ks) in enumerate(KCH):
                    pt = pt_ps.tile([128, BLK], BF16, tag="pt")
                    for s in range(NSUB):
                        gidx = blk * NSUB + s
                        cols = slice(s * SUB, (s + 1) * SUB)
                        src = (
                            cont3[:, gidx, :]
                            if k0 >= GW
                            else xall3[:, gidx, k0 : k0 + ks]
                        )
                        nc.tensor.transpose(
                            out=pt[0:ks, cols], in_=src, identity=identb[:]
                        )
                    xt = xpool.tile([128, BLK], BF16, tag=f"xt{ci}")
                    nc.scalar.copy(out=xt[0:ks, :], in_=pt[0:ks, :])
                    xt_sb.append(xt)

                # L1: h1^T = relu(W1'^T X'^T + b1)
                h1_sb = []
                for mi, (m0, ms) in enumerate(MCH1):
                    ps1 = mm_ps.tile([128, BLK], F32, tag="mm")
                    for ci, (k0, ks) in enumerate(KCH):
                        nc.tensor.matmul(
                            out=ps1[0:ms, :],
                            lhsT=w1_sb[ci][0:ks, m0 : m0 + ms],
                            rhs=xt_sb[ci][0:ks, :],
                            start=(ci == 0), stop=(ci == len(KCH) - 1),
                        )
                    h1m = hpool.tile([128, BLK], BF16, tag=f"h1m{mi}")
                    nc.scalar.activation(
                        out=h1m[0:ms, :], in_=ps1[0:ms, :], func=AF.Relu,
                        bias=b1_sb[mi][0:ms, :],
                    )
                    h1_sb.append(h1m)

                # L2: h2^T = relu(W2^T h1^T + b2)
                h2_sb = []
                for mi, (m0, ms) in enumerate(MCH2):
                    ps2 = mm_ps.tile([128, BLK], F32, tag="mm")
                    for ci, (k0, ks) in enumerate(MCH1):
                        nc.tensor.matmul(
                            out=ps2[0:ms, :],
                            lhsT=w2_sb[ci][0:ks, m0 : m0 + ms],
                            rhs=h1_sb[ci][0:ks, :],
                            start=(ci == 0), stop=(ci == len(MCH1) - 1),
                        )
                    h2m = hpool.tile([128, BLK], BF16, tag=f"h2m{mi}")
                    nc.scalar.activation(
                        out=h2m[0:ms, :], in_=ps2[0:ms, :], func=AF.Relu,
                        bias=b2_sb[mi][0:ms, :],
                    )
                    h2_sb.append(h2m)

                # out layer: y = W_out[1:]^T h2^T + w_fm*fm + b
                pso = o_ps.tile([1, BLK], F32, tag="pso")
                for ci, (k0, ks) in enumerate(MCH2):
                    nc.tensor.matmul(
                        out=pso[0:1, :],
                        lhsT=wo_sb[0:ks, ci : ci + 1],
                        rhs=h2_sb[ci][0:ks, :],
                        start=(ci == 0), stop=(ci == len(MCH2) - 1),
                    )
                pft = o_ps.tile([1, BLK], F32, tag="pft")
                for s in range(NSUB):
                    gidx = blk * NSUB + s
                    nc.tensor.transpose(
                        out=pft[0:1, s * SUB : (s + 1) * SUB],
                        in_=fmall[:, gidx : gidx + 1],
                        identity=identf[:],
                    )
                fsb = opool.tile([1, BLK], F32, tag="fsb")
                nc.scalar.copy(out=fsb[:], in_=pft[0:1, :])
                orow = opool.tile([1, BLK], F32, tag="orow")
                nc.scalar.activation(
                    out=orow[:], in_=pso[0:1, :], func=AF.Identity,
                    bias=ob_sb[0:1, :],
                )
                oout = opool.tile([1, BLK], F32, tag="oout")
                nc.vector.scalar_tensor_tensor(
                    out=oout[:], in0=fsb[:], scalar=fs_sb[0:1, 0:1], in1=orow[:],
                    op0=ALU.mult, op1=ALU.add,
                )
                nc.sync.dma_start(out=t_y[blk], in_=oout[:])

    nc.compile()
    return nc


def prep_inputs(continuous, cat_idx, W_cont, b_cont, emb_first, emb, W1, b1,
                W2, b2, W_out, b_out):
    """Host-side packing: quad table, int16 indices, selection masks."""
    emb = np.ascontiguousarray(emb, np.float32)              # [F, V, D]
    emb_first = np.ascontiguousarray(emb_first, np.float32)  # [F, V]
    rows32 = np.zeros((F, V, 32), np.float32)
    rows32[:, :, 0:D] = emb
    rows32[:, :, D] = emb_first
    ptab = rows32.reshape(F, QV, QE).astype(BFNP)

    idx = np.asarray(cat_idx).astype(np.int64)               # [B, F]

    # X' row order: [emb f-major 17-wide | cont]
    W1 = np.asarray(W1, np.float32)
    w1p = np.zeros((XW, H1), np.float32)
    for ff in range(F):
        w1p[W17 * ff : W17 * ff + D] = (
            W1[NCONT + D * ff : NCONT + D * ff + D])
    w1p[GW : GW + NCONT] = W1[0:NCONT]

    W_out = np.asarray(W_out, np.float32)
    n_wo_ch = len(MCH2)
    wo_t = np.zeros((n_wo_ch, 128), np.float32)
    wo_t.reshape(-1)[:H2] = W_out[1:, 0]
    wo = np.ascontiguousarray(wo_t.T)

    w_fm = np.float32(W_out[0, 0])
    ob = np.float32(b_out[0] + w_fm * b_cont[0])

    common = {
        "ptab": ptab,
        "w1p": w1p.astype(BFNP),
        "w2": np.ascontiguousarray(W2, np.float32).astype(BFNP),
        "b1": np.asarray(b1, np.float32).reshape(H1, 1),
        "b2": np.asarray(b2, np.float32).reshape(H2, 1),
        "wo": wo.astype(BFNP),
        "wc": np.tile(np.asarray(W_cont, np.float32).reshape(1, NCONT),
                      (128, 1)).astype(BFNP),
        "fs": np.array([[w_fm]], np.float32),
        "idb": np.eye(128, dtype=np.float32).astype(BFNP),
        "idf": np.eye(128, dtype=np.float32),
        "ob": np.array([[ob]], np.float32),
    }

    continuous = np.asarray(continuous, np.float32)
    in_maps = []
    for c in range(NCORES):
        rows = slice(c * BC, (c + 1) * BC)
        idx_c = idx[rows]                                    # [BC, F]
        qv = (idx_c >> 2).astype(np.int16)                   # [BC, F]
        sel = (idx_c & 3).astype(np.uint8)                   # [BC, F]

        # wrapped-16 int16 indices, replicated to all 128 partitions
        idx16 = np.empty((SUB, F * SUB), np.int16)
        for f in range(F):
            wrap = np.ascontiguousarray(qv[:, f].reshape(SUB, 16).T)  # [16, 128]
            idx16[:, f * SUB : (f + 1) * SUB] = np.tile(wrap, (8, 1))

        # selection masks [p, quarter, f, g]
        msk = np.zeros((SUB, 4 * F * NG), np.uint8)
        for q in (0, 1, 2, 3):
            for f in range(F):
                mg = np.ascontiguousarray(
                    (sel[:, f] == q).astype(np.uint8).reshape(NG, SUB).T)
                msk[:, q * F * NG + f * NG : q * F * NG + (f + 1) * NG] = mg

        cont3 = (continuous[rows].reshape(NG, SUB, NCONT)
                 .transpose(1, 0, 2).reshape(SUB, NG * NCONT)).astype(BFNP)

        in_maps.append({
            **common,
            "idx16": idx16,
            "msk": msk,
            "cont3": np.ascontiguousarray(cont3),
        })
    return in_maps


_NC_CACHE = {}


def kernel(**inputs) -> np.ndarray:
    if "nc" not in _NC_CACHE:
        _NC_CACHE["nc"] = build_kernel()
    nc = _NC_CACHE["nc"]
    in_maps = prep_inputs(**inputs)
    res = run_bass_kernel_spmd(nc, in_maps, core_ids=list(range(NCORES)))
    out = np.concatenate(
        [r["y"].reshape(BC, 1) for r in res.results], axis=0)
    return out.astype(np.float32)
